# revision 13
# baseline (speedup 1.0000x reference)
"""Per-batch-element scale: out[b] = x[b] * params[b].

x: (32, 1048576) f32, params: (32, 1) f32.
Data parallel across 8 NeuronCores: 4 batch rows per core. Each core's
(4, 1048576) slice is viewed as (128, 32768) — row b occupies 32
partitions, each holding a contiguous 32768-element chunk. The per-row
scalar is pre-expanded host-side to a (128, 1) tensor, so the kernel is
a streamed tensor_scalar multiply at HBM line rate.
"""

import numpy as np

import concourse.bacc as bacc
import concourse.mybir as mybir
from concourse.bass_utils import run_bass_kernel_spmd
from concourse.tile import TileContext

B = 32
T = 1 << 20
N_CORES = 8
ROWS = B // N_CORES          # 4 batch rows per core
RPP = 128 // ROWS            # 32 partitions per row
W = (ROWS * T) // 128        # 32768 elements per partition

F = 2048                     # steady-state chunk width
BUFS = 12
# Tapered chunk schedule: small chunks at the ends shrink pipeline
# fill/drain latency; 2048-wide chunks carry the steady state.
CHUNKS = [F] * (W // F)
assert sum(CHUNKS) == W

_nc_cache = None


def _build():
    global _nc_cache
    if _nc_cache is not None:
        return _nc_cache
    nc = bacc.Bacc(None, target_bir_lowering=False)
    x = nc.dram_tensor("x", [128, W], mybir.dt.float32, kind="ExternalInput")
    s = nc.dram_tensor("s", [128, 1], mybir.dt.float32, kind="ExternalInput")
    out = nc.dram_tensor("out", [128, W], mybir.dt.float32, kind="ExternalOutput")

    with TileContext(nc) as tc:
        with (
            tc.tile_pool(name="scale", bufs=1) as spool,
            tc.tile_pool(name="io", bufs=BUFS) as pool,
        ):
            st = spool.tile([128, 1], mybir.dt.float32)
            nc.sync.dma_start(out=st[:], in_=s[:])
            off = 0
            for j, f in enumerate(CHUNKS):
                t = pool.tile([128, F], mybir.dt.float32)
                ld, sto = nc.sync, nc.scalar
                ld.dma_start(out=t[:, :f], in_=x[:, off:off + f])
                nc.vector.tensor_mul(t[:, :f], t[:, :f],
                                     st[:].to_broadcast((128, f)))
                sto.dma_start(out=out[:, off:off + f], in_=t[:, :f])
                off += f
    nc.finalize()
    _nc_cache = nc
    return nc


def kernel(x: np.ndarray, params: np.ndarray, _trace: bool = False) -> np.ndarray:
    nc = _build()
    x = np.asarray(x, dtype=np.float32)
    p = np.asarray(params, dtype=np.float32).reshape(B)
    in_maps = []
    for c in range(N_CORES):
        xs = x[c * ROWS:(c + 1) * ROWS].reshape(128, W)
        ss = np.repeat(p[c * ROWS:(c + 1) * ROWS], RPP).reshape(128, 1)
        in_maps.append({"x": xs, "s": np.ascontiguousarray(ss)})
    res = run_bass_kernel_spmd(
        nc, in_maps, core_ids=list(range(N_CORES)), trace=_trace
    )
    kernel.last_result = res
    outs = [r["out"].reshape(ROWS, T) for r in res.results]
    return np.concatenate(outs, axis=0)


# revision 14
# speedup vs baseline: 1.0895x; 1.0895x over previous
"""Per-batch-element scale: out[b] = x[b] * params[b].

x: (32, 1048576) f32, params: (32, 1) f32.
Data parallel across 8 NeuronCores: 4 batch rows per core. Each core's
(4, 1048576) slice is viewed as (128, 32768) — row b occupies 32
partitions, each holding a contiguous 32768-element chunk. The per-row
scalar is pre-expanded host-side to a (128, 1) tensor, so the kernel is
a streamed tensor_scalar multiply at HBM line rate.
"""

import numpy as np

import concourse.bacc as bacc
import concourse.mybir as mybir
from concourse.bass_utils import run_bass_kernel_spmd
from concourse.tile import TileContext

B = 32
T = 1 << 20
N_CORES = 8
ROWS = B // N_CORES          # 4 batch rows per core
RPP = 128 // ROWS            # 32 partitions per row
W = (ROWS * T) // 128        # 32768 elements per partition

F = 2048                     # steady-state chunk width
BUFS = 12
# Tapered chunk schedule: small chunks at the ends shrink pipeline
# fill/drain latency; 2048-wide chunks carry the steady state.
CHUNKS = [F] * (W // F)
assert sum(CHUNKS) == W

_nc_cache = None


def _build():
    global _nc_cache
    if _nc_cache is not None:
        return _nc_cache
    nc = bacc.Bacc(None, target_bir_lowering=False)
    x = nc.dram_tensor("x", [128, W], mybir.dt.float32, kind="ExternalInput")
    s = nc.dram_tensor("s", [128, 1], mybir.dt.float32, kind="ExternalInput")
    out = nc.dram_tensor("out", [128, W], mybir.dt.float32, kind="ExternalOutput")

    with TileContext(nc) as tc:
        with (
            tc.tile_pool(name="scale", bufs=1) as spool,
            tc.tile_pool(name="io", bufs=BUFS) as pool,
        ):
            st = spool.tile([128, 1], mybir.dt.float32)
            nc.sync.dma_start(out=st[:], in_=s[:])
            off = 0
            for j, f in enumerate(CHUNKS):
                t = pool.tile([128, F], mybir.dt.float32)
                ld, sto = nc.sync, nc.scalar
                ld.dma_start(out=t[:, :f], in_=x[:, off:off + f])
                nc.vector.tensor_mul(t[:, :f], t[:, :f],
                                     st[:].to_broadcast((128, f)))
                sto.dma_start(out=out[:, off:off + f], in_=t[:, :f])
                off += f
    nc.finalize()
    _nc_cache = nc
    return nc


def kernel(x: np.ndarray, params: np.ndarray, _trace: bool = False,
           _trace_cores=None) -> np.ndarray:
    nc = _build()
    x = np.asarray(x, dtype=np.float32)
    p = np.asarray(params, dtype=np.float32).reshape(B)
    in_maps = []
    for c in range(N_CORES):
        xs = x[c * ROWS:(c + 1) * ROWS].reshape(128, W)
        ss = np.repeat(p[c * ROWS:(c + 1) * ROWS], RPP).reshape(128, 1)
        in_maps.append({"x": xs, "s": np.ascontiguousarray(ss)})
    res = run_bass_kernel_spmd(
        nc, in_maps, core_ids=list(range(N_CORES)), trace=_trace,
        trace_cores=_trace_cores,
    )
    kernel.last_result = res
    outs = [r["out"].reshape(ROWS, T) for r in res.results]
    return np.concatenate(outs, axis=0)
